# revision 25
# baseline (speedup 1.0000x reference)
"""DglGraphConvolution Trainium2 kernel — dense-adjacency matmul.

Math:  out[b] = (A_bᵀ @ (text_b @ W)) * dinv_b + bias,  dinv = 1/(deg+1)
Computed as  outᵀ = hidᵀ @ A  with  hid = text @ W,  so every matmul
uses natural layouts (no on-chip transposes):

  1. Host ships, per graph: dense adjacency counts A[src, dst] as
     fp8_e4m3 (exact small ints; 16.8 MB vs 128 MB of one-hot tiles),
     textᵀ [fin, node] bf16 (host pre-transpose), and 1/(deg+1)
     replicated across partitions (bf16).
  2. Prologue (PSUM free): hid[node, f] = textᵀ-chunkᵀ @ W for both
     graphs — lhsT = textᵀ slice, rhs = stationary W; 4 windows share
     one PSUM bank, DVE evacuates to bf16 SBUF.
  3. Main: outᵀ[f, dst] = Σ_ws hid[ws]ᵀ @ A[ws] — lhsT = hid slice
     [ns, f] (natural), rhs = fp8 adjacency slab, free dim 512; all 8
     PSUM banks accumulate one graph's full [128, 4096] result.
  4. Tail per 512-chunk: DVE multiplies PSUM by dinv, ACT adds
     per-partition bias (emits bf16), DMA out [F, N].
  5. Host transposes each graph's [F, N] result back to [N, F].

Adjacency slabs alternate between the two HWDGE queues (SP + ACT);
dinv DMAs are deferred behind the slab stream (needed only at tails);
graph 0's outputs ride the SWDGE queue.  2 graphs per core, 8 cores.
"""

import numpy as np

B, N, E, F = 16, 4096, 131072, 128
NCORES = 8
GPC = B // NCORES  # graphs per core
W = 128  # src window (partition) size
NW = N // W  # 32
NB = N // 512  # 8 psum banks / 512-wide output chunks

_cache = {}


def _build_program():
    from contextlib import ExitStack

    import concourse.bacc as bacc
    import concourse.tile as tile
    from concourse import mybir
    from concourse._compat import get_trn_type

    f32 = mybir.dt.float32
    bf16 = mybir.dt.bfloat16
    f8 = mybir.dt.float8e4

    nc = bacc.Bacc(get_trn_type() or "TRN2", target_bir_lowering=False, debug=False)

    textT_d = nc.dram_tensor("textT", [GPC, F, N], bf16, kind="ExternalInput")
    w_d = nc.dram_tensor("weightb", [F, F], bf16, kind="ExternalInput")
    bias_d = nc.dram_tensor("biascol", [F, 1], f32, kind="ExternalInput")
    dinv_d = nc.dram_tensor("dinvrep", [GPC, 128, N], bf16, kind="ExternalInput")
    adj_d = nc.dram_tensor("adj", [GPC, NW, W, N], f8, kind="ExternalInput")
    out_d = nc.dram_tensor("outT", [GPC, F, N], bf16, kind="ExternalOutput")

    with tile.TileContext(nc) as tc, ExitStack() as ctx:
        const = ctx.enter_context(tc.tile_pool(name="const", bufs=1))
        tpool = ctx.enter_context(tc.tile_pool(name="tpool", bufs=2))
        hpool = ctx.enter_context(tc.tile_pool(name="hpool", bufs=2))
        dpool = ctx.enter_context(tc.tile_pool(name="dpool", bufs=2))
        mpool = ctx.enter_context(tc.tile_pool(name="mpool", bufs=16))
        opool = ctx.enter_context(tc.tile_pool(name="opool", bufs=6))

        # textᵀ halves split across both HWDGE queues, W tucked between
        # (needed when the first hid matmul runs); bias deferred.
        textT_sb = []
        for g in range(GPC):
            t_sb = tpool.tile([128, N], bf16, tag="textT", name=f"textT{g}")
            textT_sb.append(t_sb)
        nc.sync.dma_start(textT_sb[0][:, 0 : N // 2], textT_d[0, :, 0 : N // 2])
        nc.scalar.dma_start(textT_sb[0][:, N // 2 : N], textT_d[0, :, N // 2 : N])
        w_sb = const.tile([128, F], bf16)
        nc.sync.dma_start(w_sb[:], w_d[:, :])
        nc.scalar.dma_start(textT_sb[1][:, 0 : N // 2], textT_d[1, :, 0 : N // 2])
        nc.sync.dma_start(textT_sb[1][:, N // 2 : N], textT_d[1, :, N // 2 : N])

        bias_sb = const.tile([128, 1], f32)
        nc.gpsimd.dma_start(bias_sb[:], bias_d[:, :])

        # dinv rides the idle SWDGE queue — needed only at the tails
        dinv_sb = []
        for g in range(GPC):
            d_sb = dpool.tile([128, N], bf16, tag="dinv", name=f"dinv{g}")
            nc.gpsimd.dma_start(d_sb[:], dinv_d[g])
            dinv_sb.append(d_sb)

        # prologue: hid = text @ W for both graphs while PSUM is free
        hid_sb = []
        with tc.tile_pool(name="hpsum", bufs=2, space="PSUM") as hpsum:
            for g in range(GPC):
                h_sb = hpool.tile([128, N], bf16, tag="hid", name=f"hid{g}")
                for c in range(NB):
                    hq = hpsum.tile([128, 512], f32, tag="hq", name=f"hq{g}_{c}")
                    for j in range(4):
                        ws = 4 * c + j
                        nc.tensor.matmul(
                            out=hq[:, 128 * j : 128 * (j + 1)],
                            lhsT=textT_sb[g][:, 128 * ws : 128 * (ws + 1)],
                            rhs=w_sb[:],
                            start=True,
                            stop=True,
                        )
                    nc.vector.tensor_copy(
                        h_sb[:, 512 * c : 512 * (c + 1)], hq[:]
                    )
                hid_sb.append(h_sb)

        with tc.tile_pool(name="psum", bufs=1, space="PSUM") as psum:
            for g in range(GPC):
                # outᵀ[f, dst] accumulated across all ws into 8 psum banks
                P = [
                    psum.tile([128, 512], f32, tag=f"P{b}", name=f"P{g}_{b}")
                    for b in range(NB)
                ]
                for ws in range(NW):
                    m_sb = mpool.tile([128, N], f8, tag="m", name=f"m{g}_{ws}")
                    nc.sync.dma_start(
                        m_sb[:, 0 : N // 2], adj_d[g, ws, :, 0 : N // 2]
                    )
                    nc.scalar.dma_start(
                        m_sb[:, N // 2 : N], adj_d[g, ws, :, N // 2 : N]
                    )
                    for b in range(NB):
                        nc.tensor.matmul(
                            out=P[b][:],
                            lhsT=hid_sb[g][:, 128 * ws : 128 * (ws + 1)],
                            rhs=m_sb[:, 512 * b : 512 * (b + 1)],
                            start=(ws == 0),
                            stop=(ws == NW - 1),
                        )

                for b in range(NB):
                    o1 = opool.tile([128, 512], f32, tag="o1", name=f"o1_{g}_{b}")
                    nc.vector.tensor_tensor(
                        out=o1[:],
                        in0=P[b][:],
                        in1=dinv_sb[g][:, 512 * b : 512 * (b + 1)],
                        op=mybir.AluOpType.mult,
                    )
                    o3 = opool.tile([128, 512], bf16, tag="o3", name=f"o3_{g}_{b}")
                    nc.scalar.activation(
                        o3[:],
                        o1[:],
                        mybir.ActivationFunctionType.Identity,
                        bias=bias_sb[:],
                    )
                    if g == 0:
                        eng = nc.gpsimd
                    else:
                        eng = nc.sync if b % 2 == 0 else nc.scalar
                    eng.dma_start(out_d[g, :, 512 * b : 512 * (b + 1)], o3[:])

    nc.compile()
    return nc


def kernel(text, weight, bias, edge_src, edge_dst):
    import ml_dtypes

    text = np.asarray(text, dtype=np.float32)
    weight = np.asarray(weight, dtype=np.float32)
    bias = np.asarray(bias, dtype=np.float32)
    edge_src = np.asarray(edge_src, dtype=np.int64)
    edge_dst = np.asarray(edge_dst, dtype=np.int64)

    if "nc" not in _cache:
        _cache["nc"] = _build_program()
    nc = _cache["nc"]

    fp8 = ml_dtypes.float8_e4m3
    bf = ml_dtypes.bfloat16
    lut = np.arange(64, dtype=np.float32).astype(fp8)  # exact ints through 16+

    # [B, N, F] -> [B, F, N]
    text_T = np.ascontiguousarray(text.astype(bf).transpose(0, 2, 1))
    w_bf = weight.astype(bf)
    bias_col = bias.astype(np.float32).reshape(F, 1)

    in_maps = []
    for k in range(NCORES):
        adj = np.empty((GPC, NW, W, N), dtype=fp8)
        dinv = np.empty((GPC, 128, N), dtype=bf)
        for g in range(GPC):
            b = k * GPC + g
            cnt = np.bincount(
                edge_src[b] * N + edge_dst[b], minlength=N * N
            )
            assert cnt.max() < 16, f"edge multiplicity {cnt.max()} too large"
            adj[g] = lut[cnt].reshape(NW, W, N)
            deg = np.bincount(edge_dst[b], minlength=N).astype(np.float32)
            dinv[g] = (1.0 / (deg + 1.0)).astype(bf)
        in_maps.append(
            {
                "textT": text_T[k * GPC : (k + 1) * GPC],
                "weightb": w_bf,
                "biascol": bias_col,
                "dinvrep": dinv,
                "adj": adj,
            }
        )

    _cache["in_maps"] = in_maps

    from concourse.bass_utils import run_bass_kernel_spmd

    res = run_bass_kernel_spmd(nc, in_maps, list(range(NCORES)))
    out = np.empty((B, N, F), dtype=np.float32)
    for k in range(NCORES):
        for g in range(GPC):
            out[k * GPC + g] = res.results[k]["outT"][g].astype(np.float32).T
    return out
